# revision 52
# baseline (speedup 1.0000x reference)
"""MoE-routing kernel for TRN2 (8 NeuronCores, SPMD data-parallel).

Math (see grading reference):
  perm = stable argsort(idx); sp = state[perm]
  h1[p]  = sigmoid(sp[p] @ W1[idx[perm[p]]] + b1[idx[perm[p]]])   (routed expert only)
  h_f[p] = relu(h1[p] @ W2 + b2)
  out[b] = tanh(h_f[row b] @ Wq[idx[b]][:, action[b]] + bq[idx[b], action[b]])
           (selector uses ORIGINAL-order idx/action at sorted-position row b)

Device computes, per core, over its shard of sorted rows laid out in a FIXED
group-segmented column layout:  qraw[t, c] = (h_f[c] @ WqT)[t]  for all
t = g*18+o (108 outputs).  Host does the final per-sample gather + bq + tanh.

DMA schedule: inputs stream in need-order (w1 group 0 + first xt chunk first)
across three HWDGE queues; qout streams back in column chunks during compute.
"""

import sys

sys.path.insert(0, "/opt/trn_rl_repo")

import numpy as np
import ml_dtypes

from concourse import bacc, bass, tile
from concourse import mybir
from concourse.bass_utils import run_bass_kernel_spmd

BF16 = ml_dtypes.bfloat16
FP8 = ml_dtypes.float8_e4m3  # TRN float8e4: IEEE-style, max normal 240
FMAX = 240.0

B, G, IN, F, O = 65536, 6, 512, 256, 18
NCORES = 8
T108 = G * O  # 108
ST = 512  # supertile width (columns per PSUM bank)


def _xt_chunks(n_st):
    """Input stream chunks: tiny first chunk so st0 compute starts early."""
    bounds = [0, 1, 3, 7, 12]
    bounds = [b for b in bounds if b < n_st] + [n_st]
    return list(zip(bounds, bounds[1:]))


def _out_chunks(n_st):
    """Output stream chunks: ~4 equal column ranges, first slightly larger."""
    n = min(4, n_st)
    base, rem = divmod(n_st, n)
    bounds = [0]
    for i in range(n):
        bounds.append(bounds[-1] + base + (1 if i < rem else 0))
    ch = list(zip(bounds, bounds[1:]))
    # split the final chunk down to a single st so the post-compute tail
    # transfer is as short as possible
    lo, hi = ch[-1]
    if hi - lo >= 2:
        mid = (lo + hi + 1) // 2
        ch[-1:] = [(lo, mid), (mid, hi)]
        lo, hi = ch[-1]
        if hi - lo >= 2:
            ch[-1:] = [(lo, hi - 1), (hi - 1, hi)]
    return ch


def _build_program(capf: int, rp: int, sig_scale, chunk):
    """Emit the SPMD Bass program for the fixed column layout.

    capf: per-core per-group column capacity; group g occupies columns
          [g*capf, (g+1)*capf) (last group extended to rp).
    rp:   total padded columns per core (multiple of 512).
    """
    nc = bacc.Bacc(None, target_bir_lowering=False)
    bf = mybir.dt.bfloat16
    f8 = mybir.dt.float8e4
    f32 = mybir.dt.float32

    n_st = rp // ST
    xch = _xt_chunks(n_st)
    och = _out_chunks(n_st)
    oend = {hi: (lo, hi) for (lo, hi) in och}

    # xt chunk i packs sts [lo, hi): xt{i}[p, (st-lo)*4+kc, c]
    #   = stateq_T[kc*128+p, st*512+c]
    xt_ds = [
        nc.declare_dram_parameter(f"xt{i}", [128, (hi - lo) * 4, ST], f8,
                                  isOutput=False)
        for i, (lo, hi) in enumerate(xch)
    ]
    # w1[p, g*4+fc*2+kp, r, m] = W1q[g][(kp*2+r)*128+p, fc*128+m]  (group-major)
    w1_d = nc.declare_dram_parameter("w1", [128, 4 * G, 2, 128], f8,
                                     isOutput=False)
    w2_d = nc.declare_dram_parameter("w2", [128, 512], bf, isOutput=False)
    wq_d = nc.declare_dram_parameter("wq", [128, 2 * T108], bf, isOutput=False)
    # bias: cols 0..11 = b1 (fc*6+g), cols 12..13 = b2 (jc)
    bias_d = nc.declare_dram_parameter("bias", [128, 2 * G + 2], f32,
                                       isOutput=False)
    qout_d = nc.declare_dram_parameter("qout", [T108, rp], bf, isOutput=True)

    # segment list: (group, col_start, col_end) over [0, rp), trimmed to the
    # columns actually occupied (col offsets within a group are < chunk[g];
    # the padding columns are never gathered by the host, and mm2/mm3 are
    # column-independent, so skipping mm1/sigmoid there is safe)
    segs = []
    for g in range(G):
        a = g * capf
        segs.append((g, a, a + int(chunk[g])))

    Sig = mybir.ActivationFunctionType.Sigmoid
    Cp = mybir.ActivationFunctionType.Copy
    DR = mybir.MatmulPerfMode.DoubleRow

    with tile.TileContext(nc) as tc:
        with (
            tc.tile_pool(name="wp", bufs=1) as wp,
            tc.tile_pool(name="ps", bufs=1, space="PSUM") as ps,
        ):
            w1_sb = wp.tile([128, 4 * G, 2, 128], f8)
            w2_sb = wp.tile([128, 512], bf)
            wq_sb = wp.tile([128, 2 * T108], bf)
            bias_sb = wp.tile([128, 2 * G + 2], f32)
            # persistent input + output buffers: no slot rotation -> DMAs
            # never carry a WAR wait (HW limit: 1 sync wait per DMA instr)
            xt_sb = wp.tile([128, n_st * 4, ST], f8)
            h1full = wp.tile([128, n_st * 2 * ST], bf)
            hffull = wp.tile([128, n_st * 2 * ST], bf)
            qacc = wp.tile([T108, rp], bf)

            # sync queue, in earliest-need order: st0 deps first (so the
            # first matmul is not stuck behind later transfers), then the
            # rest of w1, then the xt stream
            nc.sync.dma_start(w1_sb[:, 0:4], w1_d[:, 0:4])
            lo0, hi0 = xch[0]
            nc.sync.dma_start(xt_sb[:, lo0 * 4:hi0 * 4, :], xt_ds[0][:])
            nc.sync.dma_start(w1_sb[:, 4:4 * G], w1_d[:, 4:4 * G])
            for i, (lo, hi) in enumerate(xch[1:]):
                nc.sync.dma_start(xt_sb[:, lo * 4:hi * 4, :],
                                  xt_ds[i + 1][:])
            # small weights on the ACT queue (only SP/ACT have HWDGE) so
            # they don't delay the sync queue's xt stream
            nc.scalar.dma_start(w2_sb[:], w2_d[:])
            nc.scalar.dma_start(bias_sb[:], bias_d[:])
            nc.scalar.dma_start(wq_sb[:], wq_d[:])

            # engine warmups: absorb the bias-DMA wait so later ACT/DVE
            # instructions carry only their producer semaphore wait
            warm = wp.tile([128, 1], f32)
            warm2 = wp.tile([128, 1], f32)
            nc.scalar.activation(warm[:], bias_sb[:, 0:1], Cp, bias=0.0)
            nc.vector.tensor_copy(warm2[:], bias_sb[:, 0:1])

            for st in range(n_st):
                lo, hi = st * ST, (st + 1) * ST
                # local segments intersected with this supertile
                lsegs = []
                for (g, a, b) in segs:
                    la, lb = max(a, lo), min(b, hi)
                    if la < lb:
                        lsegs.append((g, la - lo, lb - lo))

                # mm1 (routed expert, fp8 DoubleRow: 2 k-tiles per pass)
                # + sigmoid with folded dequant scale, per 128-wide f-chunk
                hb = st * 2 * ST
                for fc in range(2):
                    h1ps = ps.tile([128, ST], f32, tag="h1ps",
                                   name="h1ps", bufs=3)
                    for (g, la, lb) in lsegs:
                        for kp in range(2):
                            nc.tensor.matmul(
                                h1ps[:, la:lb],
                                w1_sb[:, g * 4 + fc * 2 + kp, :, :],
                                xt_sb[:, st * 4 + 2 * kp:st * 4 + 2 * kp + 2,
                                      la:lb],
                                start=(kp == 0),
                                stop=(kp == 1),
                                perf_mode=DR,
                            )
                    for (g, la, lb) in lsegs:
                        nc.scalar.activation(
                            h1full[:, hb + fc * ST + la:hb + fc * ST + lb],
                            h1ps[:, la:lb],
                            Sig,
                            bias=bias_sb[:, fc * G + g:fc * G + g + 1],
                            scale=float(sig_scale[g]),
                        )

                # mm2 (shared layer); relu on DVE to unload the ACT engine
                for jc in range(2):
                    hfps = ps.tile([128, ST], f32, tag="hfps",
                                   name="hfps", bufs=3)
                    for fc in range(2):
                        wcol = (fc * 2 + jc) * 128
                        nc.tensor.matmul(
                            hfps[:],
                            w2_sb[:, wcol:wcol + 128],
                            h1full[:, hb + fc * ST:hb + (fc + 1) * ST],
                            start=(fc == 0),
                            stop=(fc == 1),
                        )
                    nc.vector.tensor_scalar(
                        hffull[:, hb + jc * ST:hb + (jc + 1) * ST],
                        hfps[:],
                        bias_sb[:, 2 * G + jc:2 * G + jc + 1],
                        0.0,
                        mybir.AluOpType.add,
                        mybir.AluOpType.max,
                    )

                # mm3: all 108 head outputs
                qps = ps.tile([T108, ST], f32, tag="qps", name="qps", bufs=2)
                for jc in range(2):
                    nc.tensor.matmul(
                        qps[:],
                        wq_sb[:, jc * T108:(jc + 1) * T108],
                        hffull[:, hb + jc * ST:hb + (jc + 1) * ST],
                        start=(jc == 0),
                        stop=(jc == 1),
                    )
                # PSUM->SBUF copy alternates ACT/DVE to balance engine load
                # (sigmoids ~26us on ACT, relu ~25us on DVE, copies ~11us);
                # last two sts stay on DVE so the final out-DMA single-waits
                if st % 2 == 0 and st < n_st - 2:
                    nc.scalar.activation(qacc[:, lo:hi], qps[:], Cp, bias=0.0)
                else:
                    nc.vector.tensor_copy(qacc[:, lo:hi], qps[:])

                # stream finished column ranges back while compute continues
                if st + 1 in oend:
                    olo, ohi = oend[st + 1]
                    nc.gpsimd.dma_start(
                        qout_d[:, olo * ST:ohi * ST],
                        qacc[:, olo * ST:ohi * ST],
                    )

    nc.finalize()  # Bacc.compile(): splits multi-wait instrs via event sems
    return nc


def _prep(state, idx, W1, b1, W2, b2, Wq):
    """Host-side: layout constants, per-core xt shards, weight transforms."""
    counts = np.bincount(idx.astype(np.int64), minlength=G)
    chunk = -(-counts // NCORES)  # ceil
    capf = 1450
    if chunk.max() > capf:
        capf = int(chunk.max())
    rp = -(-(G * capf) // ST) * ST

    gstart = np.zeros(G, dtype=np.int64)
    gstart[1:] = np.cumsum(counts)[:-1]

    perm = np.argsort(idx, kind="stable")
    ip = idx[perm].astype(np.int64)
    p = np.arange(B, dtype=np.int64)
    off = p - gstart[ip]
    m = off // chunk[ip]
    col = ip * capf + (off - m * chunk[ip])

    sx = float(np.abs(state).max()) / FMAX
    sw = np.abs(W1.astype(np.float32)).max(axis=(1, 2)) / FMAX  # per group
    sig_scale = sx * sw

    sp = (state[perm].astype(np.float32) * (1.0 / sx)).astype(FP8)
    X = np.zeros((NCORES, rp, IN), dtype=FP8)
    X[m, col] = sp
    # per-core [n_st, 128, 4, ST]: xt[st, p, kc, c] = X[st*ST+c, kc*128+p]
    n_st = rp // ST
    xt = np.ascontiguousarray(
        X.reshape(NCORES, n_st, ST, 4, 128).transpose(0, 1, 4, 3, 2)
    )
    xtc = [
        np.ascontiguousarray(
            xt[:, lo:hi].transpose(0, 2, 1, 3, 4)
            .reshape(NCORES, 128, (hi - lo) * 4, ST)
        )
        for (lo, hi) in _xt_chunks(n_st)
    ]

    W1q = (W1.astype(np.float32) / sw[:, None, None]).astype(FP8)
    # w1h[p, g*4+fc*2+kp, r, m] = W1q[g, (kp*2+r)*128+p, fc*128+m]
    w1h = np.ascontiguousarray(
        W1q.reshape(G, 2, 2, 128, 2, 128).transpose(3, 0, 4, 1, 2, 5)
        .reshape(128, 4 * G, 2, 128)
    )
    w2h = np.ascontiguousarray(
        W2.reshape(2, 128, 2, 128).transpose(1, 0, 2, 3).reshape(128, 512)
    ).astype(BF16)
    wqh = np.ascontiguousarray(
        Wq.transpose(1, 0, 2).reshape(F, T108).reshape(2, 128, T108)
        .transpose(1, 0, 2).reshape(128, 2 * T108)
    ).astype(BF16)
    b1h = np.ascontiguousarray(
        b1.reshape(G, 2, 128).transpose(2, 1, 0).reshape(128, 2 * G)
    ).astype(np.float32)
    b2h = np.ascontiguousarray(b2.reshape(2, 128).T).astype(np.float32)
    biash = np.concatenate([b1h, b2h], axis=1)

    return capf, rp, xtc, w1h, w2h, wqh, biash, sig_scale, chunk, m, col


def _run(state, action, idx, W1, b1, W2, b2, Wq, bq, trace=False):
    capf, rp, xtc, w1h, w2h, wqh, biash, sig_scale, chunk, m, col = _prep(
        state, idx, W1, b1, W2, b2, Wq
    )
    nc = _build_program(capf, rp, sig_scale, chunk)
    in_maps = []
    for c in range(NCORES):
        d = {"w1": w1h, "w2": w2h, "wq": wqh, "bias": biash}
        for i, arr in enumerate(xtc):
            d[f"xt{i}"] = arr[c]
        in_maps.append(d)
    res = run_bass_kernel_spmd(nc, in_maps, list(range(NCORES)), trace=trace)
    q_all = np.stack([res.results[c]["qout"] for c in range(NCORES)])

    act = action[:, 0].astype(np.int64)
    idx64 = idx.astype(np.int64)
    t_sel = idx64 * O + act
    out = np.tanh(
        q_all[m, t_sel, col].astype(np.float64)
        + bq[idx64, act].astype(np.float64)
    ).astype(np.float32)
    return out, res.exec_time_ns


def kernel(state, action, idx, W1, b1, W2, b2, Wq, bq):
    out, _ = _run(state, action, idx, W1, b1, W2, b2, Wq, bq, trace=False)
    return out


# revision 53
# speedup vs baseline: 1.2116x; 1.2116x over previous
"""MoE-routing kernel for TRN2 (8 NeuronCores, SPMD data-parallel).

Math (see grading reference):
  perm = stable argsort(idx); sp = state[perm]
  h1[p]  = sigmoid(sp[p] @ W1[idx[perm[p]]] + b1[idx[perm[p]]])   (routed expert only)
  h_f[p] = relu(h1[p] @ W2 + b2)
  out[b] = tanh(h_f[row b] @ Wq[idx[b]][:, action[b]] + bq[idx[b], action[b]])
           (selector uses ORIGINAL-order idx/action at sorted-position row b)

Device computes, per core, over its shard of sorted rows laid out in a FIXED
group-segmented column layout:  qraw[t, c] = (h_f[c] @ WqT)[t]  for all
t = g*18+o (108 outputs).  Host does the final per-sample gather + bq + tanh.

DMA schedule: inputs stream in need-order (w1 group 0 + first xt chunk first)
across three HWDGE queues; qout streams back in column chunks during compute.
"""

import sys

sys.path.insert(0, "/opt/trn_rl_repo")

import numpy as np
import ml_dtypes

from concourse import bacc, bass, tile
from concourse import mybir
from concourse.bass_utils import run_bass_kernel_spmd

BF16 = ml_dtypes.bfloat16
FP8 = ml_dtypes.float8_e4m3  # TRN float8e4: IEEE-style, max normal 240
FMAX = 240.0

B, G, IN, F, O = 65536, 6, 512, 256, 18
NCORES = 8
T108 = G * O  # 108
ST = 512  # supertile width (columns per PSUM bank)


def _xt_chunks(n_st):
    """Input stream chunks: tiny first chunk so st0 compute starts early."""
    bounds = [0, 1, 3, 7, 12]
    bounds = [b for b in bounds if b < n_st] + [n_st]
    return list(zip(bounds, bounds[1:]))


def _out_chunks(n_st):
    """Output stream chunks: ~4 equal column ranges, first slightly larger."""
    n = min(4, n_st)
    base, rem = divmod(n_st, n)
    bounds = [0]
    for i in range(n):
        bounds.append(bounds[-1] + base + (1 if i < rem else 0))
    ch = list(zip(bounds, bounds[1:]))
    # halve the final chunk so the post-compute tail transfer is short
    lo, hi = ch[-1]
    if hi - lo >= 2:
        mid = (lo + hi + 1) // 2
        ch[-1:] = [(lo, mid), (mid, hi)]
    return ch


def _build_program(capf: int, rp: int, sig_scale, chunk):
    """Emit the SPMD Bass program for the fixed column layout.

    capf: per-core per-group column capacity; group g occupies columns
          [g*capf, (g+1)*capf) (last group extended to rp).
    rp:   total padded columns per core (multiple of 512).
    """
    nc = bacc.Bacc(None, target_bir_lowering=False)
    bf = mybir.dt.bfloat16
    f8 = mybir.dt.float8e4
    f32 = mybir.dt.float32

    n_st = rp // ST
    xch = _xt_chunks(n_st)
    och = _out_chunks(n_st)
    oend = {hi: (lo, hi) for (lo, hi) in och}

    # xt chunk i packs sts [lo, hi): xt{i}[p, (st-lo)*4+kc, c]
    #   = stateq_T[kc*128+p, st*512+c]
    xt_ds = [
        nc.declare_dram_parameter(f"xt{i}", [128, (hi - lo) * 4, ST], f8,
                                  isOutput=False)
        for i, (lo, hi) in enumerate(xch)
    ]
    # w1[p, g*4+fc*2+kp, r, m] = W1q[g][(kp*2+r)*128+p, fc*128+m]  (group-major)
    w1_d = nc.declare_dram_parameter("w1", [128, 4 * G, 2, 128], f8,
                                     isOutput=False)
    w2_d = nc.declare_dram_parameter("w2", [128, 512], bf, isOutput=False)
    wq_d = nc.declare_dram_parameter("wq", [128, 2 * T108], bf, isOutput=False)
    # bias: cols 0..11 = b1 (fc*6+g), cols 12..13 = b2 (jc)
    bias_d = nc.declare_dram_parameter("bias", [128, 2 * G + 2], f32,
                                       isOutput=False)
    qout_d = nc.declare_dram_parameter("qout", [T108, rp], bf, isOutput=True)

    # segment list: (group, col_start, col_end) over [0, rp), trimmed to the
    # columns actually occupied (col offsets within a group are < chunk[g];
    # the padding columns are never gathered by the host, and mm2/mm3 are
    # column-independent, so skipping mm1/sigmoid there is safe)
    segs = []
    for g in range(G):
        a = g * capf
        segs.append((g, a, a + int(chunk[g])))

    Sig = mybir.ActivationFunctionType.Sigmoid
    Cp = mybir.ActivationFunctionType.Copy
    DR = mybir.MatmulPerfMode.DoubleRow

    with tile.TileContext(nc) as tc:
        with (
            tc.tile_pool(name="wp", bufs=1) as wp,
            tc.tile_pool(name="ps", bufs=1, space="PSUM") as ps,
        ):
            w1_sb = wp.tile([128, 4 * G, 2, 128], f8)
            w2_sb = wp.tile([128, 512], bf)
            wq_sb = wp.tile([128, 2 * T108], bf)
            bias_sb = wp.tile([128, 2 * G + 2], f32)
            # persistent input + output buffers: no slot rotation -> DMAs
            # never carry a WAR wait (HW limit: 1 sync wait per DMA instr)
            xt_sb = wp.tile([128, n_st * 4, ST], f8)
            h1full = wp.tile([128, n_st * 2 * ST], bf)
            hffull = wp.tile([128, n_st * 2 * ST], bf)
            qacc = wp.tile([T108, rp], bf)

            # sync queue, in earliest-need order: st0 deps first (so the
            # first matmul is not stuck behind later transfers), then the
            # rest of w1, then the xt stream
            nc.sync.dma_start(w1_sb[:, 0:4], w1_d[:, 0:4])
            lo0, hi0 = xch[0]
            nc.sync.dma_start(xt_sb[:, lo0 * 4:hi0 * 4, :], xt_ds[0][:])
            nc.sync.dma_start(w1_sb[:, 4:4 * G], w1_d[:, 4:4 * G])
            for i, (lo, hi) in enumerate(xch[1:]):
                nc.sync.dma_start(xt_sb[:, lo * 4:hi * 4, :],
                                  xt_ds[i + 1][:])
            # small weights on the ACT queue (only SP/ACT have HWDGE) so
            # they don't delay the sync queue's xt stream
            nc.scalar.dma_start(w2_sb[:], w2_d[:])
            nc.scalar.dma_start(bias_sb[:], bias_d[:])
            nc.scalar.dma_start(wq_sb[:], wq_d[:])

            # engine warmups: absorb the bias-DMA wait so later ACT/DVE
            # instructions carry only their producer semaphore wait
            warm = wp.tile([128, 1], f32)
            warm2 = wp.tile([128, 1], f32)
            nc.scalar.activation(warm[:], bias_sb[:, 0:1], Cp, bias=0.0)
            nc.vector.tensor_copy(warm2[:], bias_sb[:, 0:1])

            for st in range(n_st):
                lo, hi = st * ST, (st + 1) * ST
                # local segments intersected with this supertile
                lsegs = []
                for (g, a, b) in segs:
                    la, lb = max(a, lo), min(b, hi)
                    if la < lb:
                        lsegs.append((g, la - lo, lb - lo))

                # mm1 (routed expert, fp8 DoubleRow: 2 k-tiles per pass)
                # + sigmoid with folded dequant scale, per 128-wide f-chunk
                hb = st * 2 * ST
                for fc in range(2):
                    h1ps = ps.tile([128, ST], f32, tag="h1ps",
                                   name="h1ps", bufs=4)
                    for (g, la, lb) in lsegs:
                        for kp in range(2):
                            nc.tensor.matmul(
                                h1ps[:, la:lb],
                                w1_sb[:, g * 4 + fc * 2 + kp, :, :],
                                xt_sb[:, st * 4 + 2 * kp:st * 4 + 2 * kp + 2,
                                      la:lb],
                                start=(kp == 0),
                                stop=(kp == 1),
                                perf_mode=DR,
                            )
                    for (g, la, lb) in lsegs:
                        nc.scalar.activation(
                            h1full[:, hb + fc * ST + la:hb + fc * ST + lb],
                            h1ps[:, la:lb],
                            Sig,
                            bias=bias_sb[:, fc * G + g:fc * G + g + 1],
                            scale=float(sig_scale[g]),
                        )

                # mm2 (shared layer); relu on DVE to unload the ACT engine
                for jc in range(2):
                    hfps = ps.tile([128, ST], f32, tag="hfps",
                                   name="hfps", bufs=2)
                    for fc in range(2):
                        wcol = (fc * 2 + jc) * 128
                        nc.tensor.matmul(
                            hfps[:],
                            w2_sb[:, wcol:wcol + 128],
                            h1full[:, hb + fc * ST:hb + (fc + 1) * ST],
                            start=(fc == 0),
                            stop=(fc == 1),
                        )
                    nc.vector.tensor_scalar(
                        hffull[:, hb + jc * ST:hb + (jc + 1) * ST],
                        hfps[:],
                        bias_sb[:, 2 * G + jc:2 * G + jc + 1],
                        0.0,
                        mybir.AluOpType.add,
                        mybir.AluOpType.max,
                    )

                # mm3: all 108 head outputs
                qps = ps.tile([T108, ST], f32, tag="qps", name="qps", bufs=2)
                for jc in range(2):
                    nc.tensor.matmul(
                        qps[:],
                        wq_sb[:, jc * T108:(jc + 1) * T108],
                        hffull[:, hb + jc * ST:hb + (jc + 1) * ST],
                        start=(jc == 0),
                        stop=(jc == 1),
                    )
                # PSUM->SBUF copy alternates ACT/DVE to balance engine load
                # (sigmoids ~26us on ACT, relu ~25us on DVE, copies ~11us);
                # last two sts stay on DVE so the final out-DMA single-waits
                if st % 2 == 0 and st < n_st - 2:
                    nc.scalar.activation(qacc[:, lo:hi], qps[:], Cp, bias=0.0)
                else:
                    nc.vector.tensor_copy(qacc[:, lo:hi], qps[:])

                # stream finished column ranges back while compute continues
                if st + 1 in oend:
                    olo, ohi = oend[st + 1]
                    nc.gpsimd.dma_start(
                        qout_d[:, olo * ST:ohi * ST],
                        qacc[:, olo * ST:ohi * ST],
                    )

    nc.finalize()  # Bacc.compile(): splits multi-wait instrs via event sems
    return nc


def _prep(state, idx, W1, b1, W2, b2, Wq):
    """Host-side: layout constants, per-core xt shards, weight transforms."""
    counts = np.bincount(idx.astype(np.int64), minlength=G)
    chunk = -(-counts // NCORES)  # ceil
    capf = 1450
    if chunk.max() > capf:
        capf = int(chunk.max())
    rp = -(-(G * capf) // ST) * ST

    gstart = np.zeros(G, dtype=np.int64)
    gstart[1:] = np.cumsum(counts)[:-1]

    perm = np.argsort(idx, kind="stable")
    ip = idx[perm].astype(np.int64)
    p = np.arange(B, dtype=np.int64)
    off = p - gstart[ip]
    m = off // chunk[ip]
    col = ip * capf + (off - m * chunk[ip])

    sx = float(np.abs(state).max()) / FMAX
    sw = np.abs(W1.astype(np.float32)).max(axis=(1, 2)) / FMAX  # per group
    sig_scale = sx * sw

    sp = (state[perm].astype(np.float32) * (1.0 / sx)).astype(FP8)
    X = np.zeros((NCORES, rp, IN), dtype=FP8)
    X[m, col] = sp
    # per-core [n_st, 128, 4, ST]: xt[st, p, kc, c] = X[st*ST+c, kc*128+p]
    n_st = rp // ST
    xt = np.ascontiguousarray(
        X.reshape(NCORES, n_st, ST, 4, 128).transpose(0, 1, 4, 3, 2)
    )
    xtc = [
        np.ascontiguousarray(
            xt[:, lo:hi].transpose(0, 2, 1, 3, 4)
            .reshape(NCORES, 128, (hi - lo) * 4, ST)
        )
        for (lo, hi) in _xt_chunks(n_st)
    ]

    W1q = (W1.astype(np.float32) / sw[:, None, None]).astype(FP8)
    # w1h[p, g*4+fc*2+kp, r, m] = W1q[g, (kp*2+r)*128+p, fc*128+m]
    w1h = np.ascontiguousarray(
        W1q.reshape(G, 2, 2, 128, 2, 128).transpose(3, 0, 4, 1, 2, 5)
        .reshape(128, 4 * G, 2, 128)
    )
    w2h = np.ascontiguousarray(
        W2.reshape(2, 128, 2, 128).transpose(1, 0, 2, 3).reshape(128, 512)
    ).astype(BF16)
    wqh = np.ascontiguousarray(
        Wq.transpose(1, 0, 2).reshape(F, T108).reshape(2, 128, T108)
        .transpose(1, 0, 2).reshape(128, 2 * T108)
    ).astype(BF16)
    b1h = np.ascontiguousarray(
        b1.reshape(G, 2, 128).transpose(2, 1, 0).reshape(128, 2 * G)
    ).astype(np.float32)
    b2h = np.ascontiguousarray(b2.reshape(2, 128).T).astype(np.float32)
    biash = np.concatenate([b1h, b2h], axis=1)

    return capf, rp, xtc, w1h, w2h, wqh, biash, sig_scale, chunk, m, col


def _run(state, action, idx, W1, b1, W2, b2, Wq, bq, trace=False):
    capf, rp, xtc, w1h, w2h, wqh, biash, sig_scale, chunk, m, col = _prep(
        state, idx, W1, b1, W2, b2, Wq
    )
    nc = _build_program(capf, rp, sig_scale, chunk)
    in_maps = []
    for c in range(NCORES):
        d = {"w1": w1h, "w2": w2h, "wq": wqh, "bias": biash}
        for i, arr in enumerate(xtc):
            d[f"xt{i}"] = arr[c]
        in_maps.append(d)
    res = run_bass_kernel_spmd(nc, in_maps, list(range(NCORES)), trace=trace)
    q_all = np.stack([res.results[c]["qout"] for c in range(NCORES)])

    act = action[:, 0].astype(np.int64)
    idx64 = idx.astype(np.int64)
    t_sel = idx64 * O + act
    out = np.tanh(
        q_all[m, t_sel, col].astype(np.float64)
        + bq[idx64, act].astype(np.float64)
    ).astype(np.float32)
    return out, res.exec_time_ns


def kernel(state, action, idx, W1, b1, W2, b2, Wq, bq):
    out, _ = _run(state, action, idx, W1, b1, W2, b2, Wq, bq, trace=False)
    return out
